# revision 8
# baseline (speedup 1.0000x reference)
import os as _os
import sys as _sys

for _p in ("/opt/trn_rl_repo", "/root/.axon_site/_ro/trn_rl_repo",
           "/root/.axon_site", "/root/.axon_site/_ro/pypackages"):
    if _os.path.isdir(_p) and _p not in _sys.path:
        _sys.path.append(_p)

"""DCNv2 block kernel for TRN2 (Bass/Tile), v2.

Per-core program: one batch sample.
  x [1024, 384] -> transpose -> padded bf16 image xtp [384ch, 42*42]
  offset conv 3x3 (384->72, bf16) -> positions -> floor/frac/corner weights
  corner weights broadcast to channel partitions via DRAM-bounce DMA (bf16)
  ap_gather (d=4 packed corners, bf16) -> one TT mult + windowed reduce
  dcn matmul (K=3456, bf16) -> BN+SiLU (one Silu activation) -> 1x1 conv in
  pixel-major form (z as lhsT) -> out [1024, 384] with no output transposes.
"""

import numpy as np
from contextlib import ExitStack

import concourse.bass as bass
import concourse.tile as tile
from concourse import mybir
from concourse import library_config

F32 = mybir.dt.float32
BF16 = mybir.dt.bfloat16
I16 = mybir.dt.int16
I32 = mybir.dt.int32
ALU = mybir.AluOpType
ACTF = mybir.ActivationFunctionType

DIM, KK, G, Cg = 384, 9, 4, 96
H = W = 32
HW = 1024
PAD = 4
PH = PW = H + 2 * PAD          # 40
PHW = PH * PW                  # 1600
NT = KK                        # 9 taps
NCT = DIM // 128               # 3
NM = DIM // 128                # 3
OFFP = 100                     # offset rows: dy 0..35, dx 64..99
XOFF = 64
NPT = HW // 128                # 8
MAGIC = float(2 ** 23)

# (start, end, group) partition spans per channel tile
CT_SPANS = [
    [(0, 96, 0), (96, 128, 1)],
    [(0, 64, 1), (64, 128, 2)],
    [(0, 32, 2), (32, 128, 3)],
]


def gk_row(g, k):
    return g * KK + k


def host_prep(inputs: dict) -> dict:
    """Pure-layout host prep of weights/constants (shared by all cores)."""
    import ml_dtypes
    w_off = np.asarray(inputs["w_off"], np.float32)      # [72, 384, 3, 3]
    b_off = np.asarray(inputs["b_off"], np.float32)      # [72]
    w_dcn = np.asarray(inputs["w_dcn"], np.float32)      # [384, 384, 3, 3]
    w2 = np.asarray(inputs["w2"], np.float32)            # [384, 384]

    # offset conv rows: gk = dy rows 0..35, 36+gk = dx rows
    w_off_p = np.zeros((OFFP, DIM, 3, 3), np.float32)
    b_off_p = np.zeros((36, 2), np.float32)
    for g in range(G):
        for k in range(KK):
            w_off_p[gk_row(g, k)] = w_off[g * 18 + k * 2 + 0]
            w_off_p[XOFF + gk_row(g, k)] = w_off[g * 18 + k * 2 + 1]
            b_off_p[gk_row(g, k), 0] = b_off[g * 18 + k * 2 + 0]
            b_off_p[gk_row(g, k), 1] = b_off[g * 18 + k * 2 + 1]

    # offset conv lhsT tiles [128, 27, 72] bf16; K order = (tap, ct)
    w_offT = np.zeros((128, NT * NCT, OFFP), np.float32)
    for t in range(NT):
        ky, kx = t // 3, t % 3
        for ct in range(NCT):
            cs = ct * 128
            w_offT[:, t * NCT + ct, :] = w_off_p[:, cs:cs + 128, ky, kx].T
    w_offT = w_offT.reshape(128, NT * NCT * OFFP)

    # grid [36, 2*HW] f32, cols in rho order (rho(n) = (n%64)*16 + n//64)
    jj = np.arange(HW)
    rho = (jj % 64) * 16 + jj // 64
    yy = (np.arange(HW) // W)[rho]
    xx = (np.arange(HW) % W)[rho]
    grid_s = np.zeros((36, 2 * HW), np.float32)
    for g in range(G):
        for k in range(KK):
            grid_s[gk_row(g, k), 0:HW] = (k // 3 - 1) + yy + PAD
            grid_s[gk_row(g, k), HW:] = (k % 3 - 1) + xx + PAD

    # dcn lhsT tiles [128, 27, 384] bf16
    w_dcn_r = w_dcn.reshape(DIM, DIM, KK)
    w_dcnT = np.zeros((128, NT * NCT, DIM), np.float32)
    for t in range(NT):
        for ct in range(NCT):
            cs = ct * 128
            w_dcnT[:, t * NCT + ct, :] = w_dcn_r[:, cs:cs + 128, t].T
    w_dcnT = w_dcnT.reshape(128, NT * NCT * DIM)

    # 1x1 conv rhs tiles (pixel-major matmul): w2r[c, kt*384+o] = w2[o, kt*128+c]
    w2r = np.zeros((128, NCT, DIM), np.float32)
    for kt in range(NCT):
        w2r[:, kt, :] = w2[:, kt * 128:(kt + 1) * 128].T
    w2r = w2r.reshape(128, NCT * DIM)

    consts = {
        "w_offT": w_offT.astype(ml_dtypes.bfloat16),
        "b_off_p": b_off_p,
        "grid_s": grid_s,
        "w_dcnT": w_dcnT.astype(ml_dtypes.bfloat16),
        "w2r": w2r.astype(ml_dtypes.bfloat16),
        "b2r": np.asarray(inputs["b2"], np.float32).reshape(1, DIM).astype(ml_dtypes.bfloat16),
        "ones1": np.ones((1, 128), np.float32).astype(ml_dtypes.bfloat16),
        "ident": np.eye(128, dtype=np.float32),
        "sconst": np.tile(np.array([[MAGIC, -MAGIC, float(PW), 1.0, -1.0]],
                                   np.float32), (36, 1)),
        "bn_gamma": np.asarray(inputs["bn_gamma"], np.float32),
        "bn_beta": np.asarray(inputs["bn_beta"], np.float32),
        "bn_mean": np.asarray(inputs["bn_mean"], np.float32),
        "bn_var": np.asarray(inputs["bn_var"], np.float32),
    }
    return consts


def declare_io(nc: bass.Bass, consts: dict):
    aps = {}
    aps["x"] = nc.dram_tensor("x", [HW, DIM], F32, kind="ExternalInput").ap()
    for name, arr in consts.items():
        dt = {np.dtype("float32"): F32}.get(arr.dtype, BF16)
        aps[name] = nc.dram_tensor(name, list(arr.shape), dt, kind="ExternalInput").ap()
    aps["out"] = nc.dram_tensor("out", [HW, DIM], F32, kind="ExternalOutput").ap()
    return aps


def build(ctx: ExitStack, tc: tile.TileContext, io: dict):
    nc = tc.nc
    P = 128
    nc.gpsimd.load_library(library_config.ap_gather)

    const_pool = ctx.enter_context(tc.tile_pool(name="consts", bufs=1))
    d2_pool = ctx.enter_context(tc.tile_pool(name="d2", bufs=1))
    mid_pool = ctx.enter_context(tc.tile_pool(name="mid", bufs=1))

    # ---------- constants ----------
    w_offT = const_pool.tile([P, NT * NCT * OFFP], BF16)
    nc.sync.dma_start(w_offT[:], io["w_offT"][:])
    grid_s = const_pool.tile([36, 2 * HW], F32)
    nc.sync.dma_start(grid_s[:], io["grid_s"][:])
    w_dcnT = const_pool.tile([P, NT * NCT * DIM], BF16)
    nc.sync.dma_start(w_dcnT[:], io["w_dcnT"][:])
    w2r = const_pool.tile([P, NCT * DIM], BF16)
    nc.sync.dma_start(w2r[:], io["w2r"][:])
    b2r = const_pool.tile([1, DIM], BF16)
    nc.sync.dma_start(b2r[:], io["b2r"][:])
    ones1 = const_pool.tile([1, P], BF16)
    nc.sync.dma_start(ones1[:], io["ones1"][:])
    ident = const_pool.tile([P, P], F32)
    nc.sync.dma_start(ident[:], io["ident"][:])
    b_off_s = const_pool.tile([36, 2], F32)
    nc.sync.dma_start(b_off_s[:], io["b_off_p"][:])
    sconst = const_pool.tile([36, 5], F32)
    nc.sync.dma_start(sconst[:], io["sconst"][:])

    bnv = {}
    for vname in ("bn_gamma", "bn_beta", "bn_mean", "bn_var"):
        tl = const_pool.tile([P, NM], F32, tag=f"bn_{vname}", name=f"bn_{vname}")
        for m in range(NM):
            nc.sync.dma_start(
                tl[:, m:m + 1],
                io[vname][:].rearrange("(m p u) -> m p u", p=P, u=1)[m],
            )
        bnv[vname] = tl
    bn_scale = const_pool.tile([P, NM], F32)
    bn_shift = const_pool.tile([P, NM], F32)
    tmpv = const_pool.tile([P, NM], F32)
    nc.vector.tensor_scalar(tmpv[:], bnv["bn_var"][:], 1e-5, None, op0=ALU.add)
    nc.scalar.sqrt(tmpv[:], tmpv[:])
    nc.vector.reciprocal(tmpv[:], tmpv[:])
    nc.vector.tensor_tensor(bn_scale[:], bnv["bn_gamma"][:], tmpv[:], op=ALU.mult)
    nc.vector.tensor_tensor(tmpv[:], bnv["bn_mean"][:], bn_scale[:], op=ALU.mult)
    nc.vector.tensor_tensor(bn_shift[:], bnv["bn_beta"][:], tmpv[:], op=ALU.subtract)

    # ---------- phase 1: load x, transpose into padded bf16 image ----------
    xtp_cm = tc.tile_pool(name="xtp", bufs=1)
    xtp_pool = xtp_cm.__enter__()
    XTW = PHW + PW + 2
    xtp = [xtp_pool.tile([P, XTW], BF16, tag=f"xtp{ct}", name=f"xtp{ct}") for ct in range(NCT)]
    for ct in range(NCT):
        nc.vector.memset(xtp[ct][:], 0.0)

    with tc.tile_pool(name="ptrans", bufs=4, space="PSUM") as psum_t, \
         tc.tile_pool(name="xin", bufs=3) as xin_pool:
        for pt in range(NPT):
            xin = xin_pool.tile([P, DIM], F32)
            nc.sync.dma_start(xin[:], io["x"][pt * P:(pt + 1) * P, :])
            for ct in range(NCT):
                ps = psum_t.tile([P, P], F32)
                nc.tensor.transpose(ps[:], xin[:, ct * P:(ct + 1) * P], ident[:])
                dst = xtp[ct][:, 0:PHW].rearrange("c (y x) -> c y x", x=PW)
                dst = dst[:, PAD + pt * 4: PAD + pt * 4 + 4, PAD:PAD + W]
                nc.scalar.activation(dst, ps[:].rearrange("c (r j) -> c r j", j=W),
                                     ACTF.Copy)

    # ---------- phase 2: packed 4-corner bf16 image (split scalar/vector) ----------
    d2 = [d2_pool.tile([P, PHW, 4], BF16, tag=f"d2_{ct}", name=f"d2_{ct}") for ct in range(NCT)]
    for ct in range(NCT):
        for j, sh in enumerate((0, 1, PW, PW + 1)):
            if j == 1:
                nc.vector.tensor_copy(d2[ct][:, :, j], xtp[ct][:, sh:sh + PHW])
            else:
                nc.scalar.activation(d2[ct][:, :, j], xtp[ct][:, sh:sh + PHW], ACTF.Copy)

    # ---------- phase 3: offset conv (bf16) ----------
    off_s = mid_pool.tile([36, 2 * HW], F32, name="off_s")
    with tc.tile_pool(name="poff", bufs=1, space="PSUM") as poff_pool:
        ps_off = poff_pool.tile([OFFP, HW], F32)
        w_offT_v = w_offT[:].rearrange("r (k o) -> r k o", o=OFFP)
        n_k = NT * NCT
        for t in range(NT):
            ky, kx = t // 3, t % 3
            for ct in range(NCT):
                kt = t * NCT + ct
                rhs = xtp[ct][:, 0:PHW].rearrange("c (y x) -> c y x", x=PW)
                rhs = rhs[:, PAD - 1 + ky:PAD - 1 + ky + H, PAD - 1 + kx:PAD - 1 + kx + W]
                rhs = rhs.rearrange("c y (xh p) -> c p y xh", p=16)
                for nh in range(2):
                    nc.tensor.matmul(
                        ps_off[:, nh * 512:(nh + 1) * 512],
                        w_offT_v[:, kt, :],
                        rhs[:, nh * 8:(nh + 1) * 8, :, :],
                        start=(kt == 0), stop=(kt == n_k - 1),
                    )
        nc.scalar.activation(off_s[:, 0:HW], ps_off[0:36, :], ACTF.Identity,
                             bias=b_off_s[:, 0:1])
        nc.scalar.activation(off_s[:, HW:], ps_off[XOFF:XOFF + 36, :], ACTF.Identity,
                             bias=b_off_s[:, 1:2])
    xtp_cm.__exit__(None, None, None)

    # ---------- phase 4: positions, indices ----------
    W2 = 2 * HW
    small_cm = tc.tile_pool(name="small", bufs=1)
    small_pool = small_cm.__enter__()
    pos = small_pool.tile([36, W2], F32, name="pos")
    nc.vector.tensor_tensor(pos[:], off_s[:], grid_s[:], op=ALU.add)
    rnd = small_pool.tile([36, W2], F32, name="rnd")
    nc.scalar.add(rnd[:], pos[:], sconst[:, 0:1])
    nc.scalar.add(rnd[:], rnd[:], sconst[:, 1:2])
    cmp = small_pool.tile([36, W2], F32, name="cmp")
    nc.vector.tensor_tensor(cmp[:], rnd[:], pos[:], op=ALU.is_gt)
    flr = small_pool.tile([36, W2], F32, name="flr")
    nc.vector.tensor_tensor(flr[:], rnd[:], cmp[:], op=ALU.subtract)
    nc.vector.tensor_scalar(flr[:, 0:HW], flr[:, 0:HW], 0.0, float(PH - 2),
                            op0=ALU.max, op1=ALU.min)
    nc.vector.tensor_scalar(flr[:, HW:], flr[:, HW:], 0.0, float(PW - 2),
                            op0=ALU.max, op1=ALU.min)

    # indices first (unblocks phase 5 / gathers early)
    qf = small_pool.tile([36, HW], F32, name="qf")
    nc.scalar.mul(qf[:], flr[:, 0:HW], sconst[:, 2:3])
    nc.vector.tensor_tensor(qf[:], qf[:], flr[:, HW:], op=ALU.add)
    qi32 = small_pool.tile([36, HW], I32, name="qi32")
    nc.vector.tensor_copy(qi32[:], qf[:])
    qi16 = small_pool.tile([36, HW], I16, name="qi16")
    nc.vector.tensor_copy(qi16[:], qi32[:])

    # ---------- phase 5a: wrap indices via DRAM bounce ----------
    dram_pool = ctx.enter_context(tc.tile_pool(name="qdram", bufs=1, space="DRAM"))
    qa_dram = dram_pool.tile([36, HW], I16, name="qa_dram")
    nc.sync.dma_start(qa_dram[:], qi16[:])
    widx = [mid_pool.tile([P, NT * 64], I16, tag=f"widx{ct}", name=f"widx{ct}")
            for ct in range(NCT)]
    for ct in range(NCT):
        for cb in range(8):
            g = (ct * 128 + cb * 16) // Cg
            dst = widx[ct][cb * 16:(cb + 1) * 16, :].rearrange("p (t s) -> p t s", s=64)
            srcv = qa_dram[gk_row(g, 0):gk_row(g, 0) + NT, :]
            srcv = srcv.rearrange("t (p s) -> p t s", p=16)
            nc.sync.dma_start(dst, srcv)

    # ---------- phase 4b: corner weights, packed in gather output order ----------
    frac = small_pool.tile([36, W2], F32, name="frac")
    nc.vector.tensor_tensor(frac[:], pos[:], flr[:], op=ALU.subtract)
    gyx = small_pool.tile([36, W2], F32, name="gyx")
    nc.scalar.activation(gyx[:], frac[:], ACTF.Identity,
                         bias=sconst[:, 3:4], scale=sconst[:, 4:5])
    # wc_packed[g, m, j]: weights for gather-output col m (m-order); the
    # source cols are n-ordered with n = (m%16)*64 + m//16.
    wc_packed = small_pool.tile([36, HW * 4], BF16, name="wc_packed")
    wcp_v = wc_packed[:].rearrange("g (r s j) -> g s r j", r=64, s=16, j=4)
    fy = frac[:, 0:HW].rearrange("g (s r) -> g s r", s=16)
    fx = frac[:, HW:].rearrange("g (s r) -> g s r", s=16)
    gy = gyx[:, 0:HW].rearrange("g (s r) -> g s r", s=16)
    gx = gyx[:, HW:].rearrange("g (s r) -> g s r", s=16)
    nc.vector.tensor_tensor(wcp_v[:, :, :, 0], gy, gx, op=ALU.mult)
    nc.vector.tensor_tensor(wcp_v[:, :, :, 1], gy, fx, op=ALU.mult)
    nc.vector.tensor_tensor(wcp_v[:, :, :, 2], fy, gx, op=ALU.mult)
    nc.vector.tensor_tensor(wcp_v[:, :, :, 3], fy, fx, op=ALU.mult)

    # ---------- phase 5b: weight table to DRAM for broadcast ----------
    wc_dram = dram_pool.tile([36, HW * 4], BF16, name="wc_dram")
    nc.sync.dma_start(wc_dram[:], wc_packed[:])
    small_cm.__exit__(None, None, None)

    # ---------- phase 6+7: gather, weight, reduce, dcn matmul ----------
    # ct-outer / tap-triple loop: one pw DMA pair per (ct, T) covers 3 taps
    # with 24KB-per-partition descriptors (descriptor-count is the DMA floor).
    w_dcnT_v = w_dcnT[:].rearrange("r (k o) -> r k o", o=DIM)
    wc_view = wc_dram[:].rearrange("(g x) f -> g (x f)", g=G)
    z = [mid_pool.tile([P, HW], BF16, tag=f"z{m}", name=f"z{m}") for m in range(NM)]
    W4 = HW * 4
    with tc.tile_pool(name="pacc", bufs=1, space="PSUM") as pacc_pool, \
         tc.tile_pool(name="gaP", bufs=3) as ga_pool, \
         tc.tile_pool(name="pwP", bufs=2) as pw_pool, \
         tc.tile_pool(name="prP", bufs=2) as pr_pool, \
         tc.tile_pool(name="saP", bufs=2) as sa_pool, \
         tc.tile_pool(name="smpP", bufs=2) as smp_pool:
        accs = [pacc_pool.tile([P, HW], F32, tag=f"pa{m}", name=f"pa{m}")
                for m in range(NM)]
        for ct in range(NCT):
            for T in range(3):
                pw3 = pw_pool.tile([P, 3 * W4], BF16, tag="pw3", name="pw3")
                for (p0, p1, g) in CT_SPANS[ct]:
                    nc.sync.dma_start(
                        pw3[p0:p1, :],
                        wc_view[g:g + 1, T * 3 * W4:(T + 1) * 3 * W4].broadcast_to(
                            [p1 - p0, 3 * W4]),
                    )
                for ti in range(3):
                    t = T * 3 + ti
                    kt = t * NCT + ct
                    gA = ga_pool.tile([P, HW, 4], BF16, tag="gA", name="gA")
                    wsl = widx[ct][:].rearrange("p (t s) -> p t s", s=64)[:, t, :]
                    nc.gpsimd.ap_gather(gA[:], d2[ct][:], wsl,
                                        channels=P, num_elems=PHW, d=4, num_idxs=HW)
                    prod = pr_pool.tile([P, W4], BF16, tag="prod", name="prod")
                    nc.vector.tensor_tensor(
                        prod[:], gA[:].rearrange("c m j -> c (m j)"),
                        pw3[:, ti * W4:(ti + 1) * W4], op=ALU.mult)
                    prv = prod[:].rearrange("c (m j) -> c m j", j=4)
                    sa = sa_pool.tile([P, HW, 2], BF16, tag="sa", name="sa")
                    nc.vector.tensor_tensor(sa[:], prv[:, :, 0:2], prv[:, :, 2:4],
                                            op=ALU.add)
                    smp = smp_pool.tile([P, HW], BF16, tag="smp", name="smp")
                    nc.vector.tensor_tensor(smp[:], sa[:, :, 0], sa[:, :, 1],
                                            op=ALU.add)
                    first = (ct == 0 and t == 0)
                    last = (ct == NCT - 1 and t == NT - 1)
                    for m in range(NM):
                        for nh in range(2):
                            nc.tensor.matmul(
                                accs[m][:, nh * 512:(nh + 1) * 512],
                                w_dcnT_v[:, kt, m * P:(m + 1) * P],
                                smp[:, nh * 512:(nh + 1) * 512],
                                start=first, stop=last,
                            )
        # BN + SiLU in one activation per output tile
        for m in range(NM):
            nc.scalar.activation(z[m][:], accs[m][:], ACTF.Silu,
                                 bias=bn_shift[:, m:m + 1], scale=bn_scale[:, m:m + 1])

    # ---------- phase 8: 1x1 conv, pixel-major (output needs no transpose) ----------
    w2r_v = w2r[:].rearrange("c (k o) -> c k o", o=DIM)
    with tc.tile_pool(name="p8", bufs=3, space="PSUM") as p8_pool, \
         tc.tile_pool(name="osb", bufs=3) as osb_pool:
        for pt in range(NPT):
            ps = p8_pool.tile([P, DIM], F32)
            for kt in range(NCT):
                nc.tensor.matmul(
                    ps[:], z[kt][:, pt * P:(pt + 1) * P], w2r_v[:, kt, :],
                    start=(kt == 0), stop=False,
                )
            nc.tensor.matmul(ps[:], ones1[0:1, :], b2r[0:1, :],
                             start=False, stop=True)
            osb = osb_pool.tile([P, DIM], F32, tag="osb", name="osb")
            nc.scalar.activation(osb[:], ps[:], ACTF.Copy)
            nc.sync.dma_start(io["out"][pt * P:(pt + 1) * P, :], osb[:])


# ======================================================================
# SPMD entry point: full inputs in, full output out (8 cores, batch-parallel)
# ======================================================================

_PROGRAM_CACHE = {}


def _get_program(consts):
    key = "dcn2"
    if key not in _PROGRAM_CACHE:
        import concourse.bacc as bacc
        nc = bacc.Bacc("TRN2", target_bir_lowering=False, debug=False)
        io = declare_io(nc, consts)
        with tile.TileContext(nc) as tc:
            with ExitStack() as ctx:
                build(ctx, tc, io)
        nc.compile()
        _PROGRAM_CACHE[key] = nc
    return _PROGRAM_CACHE[key]


def kernel(**inputs) -> np.ndarray:
    from concourse.bass_utils import run_bass_kernel_spmd

    x = np.ascontiguousarray(np.asarray(inputs["x"], np.float32))
    B = x.shape[0]
    assert x.shape == (B, HW, DIM), x.shape
    consts = host_prep(inputs)
    nc = _get_program(consts)
    n_cores = 8
    reps = []
    for i in range(n_cores):
        m = {"x": x[i % B]}
        m.update(consts)
        reps.append(m)
    res = run_bass_kernel_spmd(nc, reps, list(range(n_cores)))
    out = np.stack([np.asarray(res.results[i]["out"], np.float32)
                    for i in range(B)], axis=0)
    return out


# revision 13
# speedup vs baseline: 1.2904x; 1.2904x over previous
import os as _os
import sys as _sys

for _p in ("/opt/trn_rl_repo", "/root/.axon_site/_ro/trn_rl_repo",
           "/root/.axon_site", "/root/.axon_site/_ro/pypackages"):
    if _os.path.isdir(_p) and _p not in _sys.path:
        _sys.path.append(_p)

"""DCNv2 block kernel for TRN2 (Bass/Tile), v2.

Per-core program: one batch sample.
  x [1024, 384] -> transpose -> padded bf16 image xtp [384ch, 42*42]
  offset conv 3x3 (384->72, bf16) -> positions -> floor/frac/corner weights
  corner weights broadcast to channel partitions via DRAM-bounce DMA (bf16)
  ap_gather (d=4 packed corners, bf16) -> one TT mult + windowed reduce
  dcn matmul (K=3456, bf16) -> BN+SiLU (one Silu activation) -> 1x1 conv in
  pixel-major form (z as lhsT) -> out [1024, 384] with no output transposes.
"""

import numpy as np
from contextlib import ExitStack

import concourse.bass as bass
import concourse.tile as tile
from concourse import mybir
from concourse import library_config

F32 = mybir.dt.float32
BF16 = mybir.dt.bfloat16
I16 = mybir.dt.int16
I32 = mybir.dt.int32
ALU = mybir.AluOpType
ACTF = mybir.ActivationFunctionType

DIM, KK, G, Cg = 384, 9, 4, 96
H = W = 32
HW = 1024
PAD = 4
PH = PW = H + 2 * PAD          # 40
PHW = PH * PW                  # 1600
NT = KK                        # 9 taps
NCT = DIM // 128               # 3
NM = DIM // 128                # 3
OFFP = 100                     # offset rows: dy 0..35, dx 64..99
XOFF = 64
NPT = HW // 128                # 8
MAGIC = float(2 ** 23)

# (start, end, group) partition spans per channel tile
CT_SPANS = [
    [(0, 96, 0), (96, 128, 1)],
    [(0, 64, 1), (64, 128, 2)],
    [(0, 32, 2), (32, 128, 3)],
]


def gk_row(g, k):
    return g * KK + k


def host_prep(inputs: dict) -> dict:
    """Pure-layout host prep of weights/constants (shared by all cores)."""
    import ml_dtypes
    w_off = np.asarray(inputs["w_off"], np.float32)      # [72, 384, 3, 3]
    b_off = np.asarray(inputs["b_off"], np.float32)      # [72]
    w_dcn = np.asarray(inputs["w_dcn"], np.float32)      # [384, 384, 3, 3]
    w2 = np.asarray(inputs["w2"], np.float32)            # [384, 384]

    # offset conv rows: gk = dy rows 0..35, 36+gk = dx rows
    w_off_p = np.zeros((OFFP, DIM, 3, 3), np.float32)
    b_off_p = np.zeros((36, 2), np.float32)
    for g in range(G):
        for k in range(KK):
            w_off_p[gk_row(g, k)] = w_off[g * 18 + k * 2 + 0]
            w_off_p[XOFF + gk_row(g, k)] = w_off[g * 18 + k * 2 + 1]
            b_off_p[gk_row(g, k), 0] = b_off[g * 18 + k * 2 + 0]
            b_off_p[gk_row(g, k), 1] = b_off[g * 18 + k * 2 + 1]

    # offset conv lhsT tiles [128, 27, 72] bf16; K order = (tap, ct)
    w_offT = np.zeros((128, NT * NCT, OFFP), np.float32)
    for t in range(NT):
        ky, kx = t // 3, t % 3
        for ct in range(NCT):
            cs = ct * 128
            w_offT[:, t * NCT + ct, :] = w_off_p[:, cs:cs + 128, ky, kx].T
    w_offT = w_offT.reshape(128, NT * NCT * OFFP)

    # grid [36, 2*HW] f32, cols in rho order (rho(n) = (n%64)*16 + n//64)
    jj = np.arange(HW)
    rho = (jj % 64) * 16 + jj // 64
    yy = (np.arange(HW) // W)[rho]
    xx = (np.arange(HW) % W)[rho]
    grid_s = np.zeros((36, 2 * HW), np.float32)
    for g in range(G):
        for k in range(KK):
            grid_s[gk_row(g, k), 0:HW] = (k // 3 - 1) + yy + PAD
            grid_s[gk_row(g, k), HW:] = (k % 3 - 1) + xx + PAD

    # dcn lhsT tiles [128, 9*3, 384] bf16, rows follow the packed-channel
    # partition map: partition P holds channels (P//32)*96 + ich*32 + P%32.
    w_dcn_r = w_dcn.reshape(DIM, DIM, KK)
    PP = np.arange(128)
    w_dcn3 = np.zeros((128, NT * 3, DIM), np.float32)
    for t in range(NT):
        for ich in range(3):
            cmap = (PP // 32) * 96 + ich * 32 + (PP % 32)
            w_dcn3[:, t * 3 + ich, :] = w_dcn_r[:, cmap, t].T
    w_dcn3 = w_dcn3.reshape(128, NT * 3 * DIM)

    # 1x1 conv rhs tiles (pixel-major matmul): w2r[c, kt*384+o] = w2[o, kt*128+c]
    w2r = np.zeros((128, NCT, DIM), np.float32)
    for kt in range(NCT):
        w2r[:, kt, :] = w2[:, kt * 128:(kt + 1) * 128].T
    w2r = w2r.reshape(128, NCT * DIM)

    consts = {
        "w_offT": w_offT.astype(ml_dtypes.bfloat16),
        "b_off_p": b_off_p,
        "grid_s": grid_s,
        "w_dcn3": w_dcn3.astype(ml_dtypes.bfloat16),
        "w2r": w2r.astype(ml_dtypes.bfloat16),
        "b2r": np.asarray(inputs["b2"], np.float32).reshape(1, DIM).astype(ml_dtypes.bfloat16),
        "ones1": np.ones((1, 128), np.float32).astype(ml_dtypes.bfloat16),
        "ident": np.eye(128, dtype=np.float32),
        "ident_bf": np.eye(128, dtype=np.float32).astype(ml_dtypes.bfloat16),
        "sconst": np.tile(np.array([[MAGIC, -MAGIC, float(PW), 1.0, -1.0]],
                                   np.float32), (36, 1)),
        "bn_gamma": np.asarray(inputs["bn_gamma"], np.float32),
        "bn_beta": np.asarray(inputs["bn_beta"], np.float32),
        "bn_mean": np.asarray(inputs["bn_mean"], np.float32),
        "bn_var": np.asarray(inputs["bn_var"], np.float32),
    }
    return consts


def declare_io(nc: bass.Bass, consts: dict):
    aps = {}
    aps["x"] = nc.dram_tensor("x", [HW, DIM], F32, kind="ExternalInput").ap()
    for name, arr in consts.items():
        dt = {np.dtype("float32"): F32}.get(arr.dtype, BF16)
        aps[name] = nc.dram_tensor(name, list(arr.shape), dt, kind="ExternalInput").ap()
    aps["out"] = nc.dram_tensor("out", [HW, DIM], F32, kind="ExternalOutput").ap()
    return aps


def build(ctx: ExitStack, tc: tile.TileContext, io: dict):
    nc = tc.nc
    P = 128
    nc.gpsimd.load_library(library_config.ap_gather)

    const_pool = ctx.enter_context(tc.tile_pool(name="consts", bufs=1))
    d2_pool = ctx.enter_context(tc.tile_pool(name="d2", bufs=1))
    mid_pool = ctx.enter_context(tc.tile_pool(name="mid", bufs=1))

    # ---------- constants ----------
    w_offT = const_pool.tile([P, NT * NCT * OFFP], BF16)
    nc.sync.dma_start(w_offT[:], io["w_offT"][:])
    grid_s = const_pool.tile([36, 2 * HW], F32)
    nc.sync.dma_start(grid_s[:], io["grid_s"][:])
    w_dcn3 = const_pool.tile([P, NT * 3 * DIM], BF16)
    nc.sync.dma_start(w_dcn3[:], io["w_dcn3"][:])
    w2r = const_pool.tile([P, NCT * DIM], BF16)
    nc.sync.dma_start(w2r[:], io["w2r"][:])
    b2r = const_pool.tile([1, DIM], BF16)
    nc.sync.dma_start(b2r[:], io["b2r"][:])
    ones1 = const_pool.tile([1, P], BF16)
    nc.sync.dma_start(ones1[:], io["ones1"][:])
    ident = const_pool.tile([P, P], F32)
    nc.sync.dma_start(ident[:], io["ident"][:])
    identb = const_pool.tile([P, P], BF16)
    nc.sync.dma_start(identb[:], io["ident_bf"][:])
    b_off_s = const_pool.tile([36, 2], F32)
    nc.sync.dma_start(b_off_s[:], io["b_off_p"][:])
    sconst = const_pool.tile([36, 5], F32)
    nc.sync.dma_start(sconst[:], io["sconst"][:])

    bnv = {}
    for vname in ("bn_gamma", "bn_beta", "bn_mean", "bn_var"):
        tl = const_pool.tile([P, NM], F32, tag=f"bn_{vname}", name=f"bn_{vname}")
        for m in range(NM):
            nc.sync.dma_start(
                tl[:, m:m + 1],
                io[vname][:].rearrange("(m p u) -> m p u", p=P, u=1)[m],
            )
        bnv[vname] = tl
    bn_scale = const_pool.tile([P, NM], F32)
    bn_shift = const_pool.tile([P, NM], F32)
    tmpv = const_pool.tile([P, NM], F32)
    nc.vector.tensor_scalar(tmpv[:], bnv["bn_var"][:], 1e-5, None, op0=ALU.add)
    nc.scalar.sqrt(tmpv[:], tmpv[:])
    nc.vector.reciprocal(tmpv[:], tmpv[:])
    nc.vector.tensor_tensor(bn_scale[:], bnv["bn_gamma"][:], tmpv[:], op=ALU.mult)
    nc.vector.tensor_tensor(tmpv[:], bnv["bn_mean"][:], bn_scale[:], op=ALU.mult)
    nc.vector.tensor_tensor(bn_shift[:], bnv["bn_beta"][:], tmpv[:], op=ALU.subtract)

    # ---------- phase 1: load x, transpose into padded bf16 image ----------
    xtp_cm = tc.tile_pool(name="xtp", bufs=1)
    xtp_pool = xtp_cm.__enter__()
    XTW = PHW + PW + 2
    xtp = [xtp_pool.tile([P, XTW], BF16, tag=f"xtp{ct}", name=f"xtp{ct}") for ct in range(NCT)]
    for ct in range(NCT):
        nc.vector.memset(xtp[ct][:], 0.0)

    with tc.tile_pool(name="ptrans", bufs=4, space="PSUM") as psum_t, \
         tc.tile_pool(name="xin", bufs=3) as xin_pool:
        for pt in range(NPT):
            xin = xin_pool.tile([P, DIM], F32)
            nc.sync.dma_start(xin[:], io["x"][pt * P:(pt + 1) * P, :])
            for ct in range(NCT):
                ps = psum_t.tile([P, P], F32)
                nc.tensor.transpose(ps[:], xin[:, ct * P:(ct + 1) * P], ident[:])
                dst = xtp[ct][:, 0:PHW].rearrange("c (y x) -> c y x", x=PW)
                dst = dst[:, PAD + pt * 4: PAD + pt * 4 + 4, PAD:PAD + W]
                nc.scalar.activation(dst, ps[:].rearrange("c (r j) -> c r j", j=W),
                                     ACTF.Copy)

    # ---------- phase 2: channel-packed 4-corner bf16 image ----------
    # d3[P, q, ich*4+j] = x[(P//32)*96 + ich*32 + P%32, q + sh_j]; one gather
    # per tap then serves all 384 channels (3 per partition).
    d3 = d2_pool.tile([P, PHW, 12], BF16, name="d3")
    CW = 469
    SHJ = (0, 1, PW, PW + 1)
    with tc.tile_pool(name="p3b", bufs=3, space="PSUM") as p3b_pool:
        for ich in range(3):
            for ck in range(4):
                q0 = ck * CW
                cw = min(CW, PHW - q0)
                psA = p3b_pool.tile([96, 512], F32, tag="psA", name="psA")
                psB = p3b_pool.tile([32, 512], F32, tag="psB", name="psB")
                for g in range(G):
                    c0 = g * 96 + ich * 32
                    T, off = c0 // 128, c0 % 128
                    dst = (psA[g * 32:(g + 1) * 32, 0:cw + PW + 2] if g < 3
                           else psB[:, 0:cw + PW + 2])
                    nc.tensor.matmul(
                        dst,
                        identb[:, off:off + 32],
                        xtp[T][:, q0:q0 + cw + PW + 2],
                        start=True, stop=True,
                    )
                for j, sh in enumerate(SHJ):
                    nc.scalar.activation(d3[0:96, q0:q0 + cw, ich * 4 + j],
                                         psA[:, sh:sh + cw], ACTF.Copy)
                    nc.scalar.activation(d3[96:128, q0:q0 + cw, ich * 4 + j],
                                         psB[:, sh:sh + cw], ACTF.Copy)

    # ---------- phase 3: offset conv (bf16) ----------
    off_s = mid_pool.tile([36, 2 * HW], F32, name="off_s")
    with tc.tile_pool(name="poff", bufs=1, space="PSUM") as poff_pool:
        ps_off = poff_pool.tile([OFFP, HW], F32)
        w_offT_v = w_offT[:].rearrange("r (k o) -> r k o", o=OFFP)
        n_k = NT * NCT
        for t in range(NT):
            ky, kx = t // 3, t % 3
            for ct in range(NCT):
                kt = t * NCT + ct
                rhs = xtp[ct][:, 0:PHW].rearrange("c (y x) -> c y x", x=PW)
                rhs = rhs[:, PAD - 1 + ky:PAD - 1 + ky + H, PAD - 1 + kx:PAD - 1 + kx + W]
                rhs = rhs.rearrange("c y (xh p) -> c p y xh", p=16)
                for nh in range(2):
                    nc.tensor.matmul(
                        ps_off[:, nh * 512:(nh + 1) * 512],
                        w_offT_v[:, kt, :],
                        rhs[:, nh * 8:(nh + 1) * 8, :, :],
                        start=(kt == 0), stop=(kt == n_k - 1),
                    )
        nc.scalar.activation(off_s[:, 0:HW], ps_off[0:36, :], ACTF.Identity,
                             bias=b_off_s[:, 0:1])
        nc.scalar.activation(off_s[:, HW:], ps_off[XOFF:XOFF + 36, :], ACTF.Identity,
                             bias=b_off_s[:, 1:2])
    xtp_cm.__exit__(None, None, None)

    # ---------- phase 4: positions, indices ----------
    W2 = 2 * HW
    small_cm = tc.tile_pool(name="small", bufs=1)
    small_pool = small_cm.__enter__()
    pos = small_pool.tile([36, W2], F32, name="pos")
    nc.vector.tensor_tensor(pos[:], off_s[:], grid_s[:], op=ALU.add)
    rnd = small_pool.tile([36, W2], F32, name="rnd")
    nc.scalar.add(rnd[:], pos[:], sconst[:, 0:1])
    nc.scalar.add(rnd[:], rnd[:], sconst[:, 1:2])
    cmp = small_pool.tile([36, W2], F32, name="cmp")
    nc.vector.tensor_tensor(cmp[:], rnd[:], pos[:], op=ALU.is_gt)
    flr = small_pool.tile([36, W2], F32, name="flr")
    nc.vector.tensor_tensor(flr[:], rnd[:], cmp[:], op=ALU.subtract)
    nc.vector.tensor_scalar(flr[:, 0:HW], flr[:, 0:HW], 0.0, float(PH - 2),
                            op0=ALU.max, op1=ALU.min)
    nc.vector.tensor_scalar(flr[:, HW:], flr[:, HW:], 0.0, float(PW - 2),
                            op0=ALU.max, op1=ALU.min)

    # indices first (unblocks phase 5 / gathers early)
    qf = small_pool.tile([36, HW], F32, name="qf")
    nc.scalar.mul(qf[:], flr[:, 0:HW], sconst[:, 2:3])
    nc.vector.tensor_tensor(qf[:], qf[:], flr[:, HW:], op=ALU.add)
    qi32 = small_pool.tile([36, HW], I32, name="qi32")
    nc.vector.tensor_copy(qi32[:], qf[:])
    qi16 = small_pool.tile([36, HW], I16, name="qi16")
    nc.vector.tensor_copy(qi16[:], qi32[:])

    # ---------- phase 5a: wrap indices via DRAM bounce ----------
    dram_pool = ctx.enter_context(tc.tile_pool(name="qdram", bufs=1, space="DRAM"))
    qa_dram = dram_pool.tile([36, HW], I16, name="qa_dram")
    nc.sync.dma_start(qa_dram[:], qi16[:])
    widx = mid_pool.tile([P, NT * 64], I16, name="widx")
    for cb in range(8):
        g = cb // 2
        dst = widx[cb * 16:(cb + 1) * 16, :].rearrange("p (t s) -> p t s", s=64)
        srcv = qa_dram[gk_row(g, 0):gk_row(g, 0) + NT, :]
        srcv = srcv.rearrange("t (p s) -> p t s", p=16)
        nc.sync.dma_start(dst, srcv)

    # ---------- phase 4b: corner weights, packed in gather output order ----------
    frac = small_pool.tile([36, W2], F32, name="frac")
    nc.vector.tensor_tensor(frac[:], pos[:], flr[:], op=ALU.subtract)
    gyx = small_pool.tile([36, W2], F32, name="gyx")
    nc.scalar.activation(gyx[:], frac[:], ACTF.Identity,
                         bias=sconst[:, 3:4], scale=sconst[:, 4:5])
    # wc_packed[g, m, ich*4+j]: corner weights for gather-output col m
    # (m-order), replicated across the 3 packed channels; source cols are
    # n-ordered with n = (m%16)*64 + m//16.
    wc_packed = small_pool.tile([36, HW * 12], BF16, name="wc_packed")
    wcp_v = wc_packed[:].rearrange("g (r s i j) -> g s r i j", r=64, s=16, i=3, j=4)
    fy = frac[:, 0:HW].rearrange("g (s r) -> g s r", s=16)
    fx = frac[:, HW:].rearrange("g (s r) -> g s r", s=16)
    gy = gyx[:, 0:HW].rearrange("g (s r) -> g s r", s=16)
    gx = gyx[:, HW:].rearrange("g (s r) -> g s r", s=16)
    for i in range(3):
        nc.vector.tensor_tensor(wcp_v[:, :, :, i, 0], gy, gx, op=ALU.mult)
        nc.vector.tensor_tensor(wcp_v[:, :, :, i, 1], gy, fx, op=ALU.mult)
        nc.vector.tensor_tensor(wcp_v[:, :, :, i, 2], fy, gx, op=ALU.mult)
        nc.vector.tensor_tensor(wcp_v[:, :, :, i, 3], fy, fx, op=ALU.mult)

    # ---------- phase 5b: weight table to DRAM for broadcast ----------
    wc_dram = dram_pool.tile([36, HW * 12], BF16, name="wc_dram")
    nc.sync.dma_start(wc_dram[:], wc_packed[:])
    small_cm.__exit__(None, None, None)

    # ---------- phase 6+7: gather, weight, reduce, dcn matmul ----------
    # One packed gather per (tap, pixel-half) serves all 384 channels.
    w_dcn3_v = w_dcn3[:].rearrange("r (k o) -> r k o", o=DIM)
    z = [mid_pool.tile([P, HW], BF16, tag=f"z{m}", name=f"z{m}") for m in range(NM)]
    G_SPANS = [(0, 32, 0), (32, 64, 1), (64, 96, 2), (96, 128, 3)]
    HWD = 512 * 12
    with tc.tile_pool(name="pacc", bufs=1, space="PSUM") as pacc_pool, \
         tc.tile_pool(name="gaP", bufs=2) as ga_pool, \
         tc.tile_pool(name="pwP", bufs=2) as pw_pool, \
         tc.tile_pool(name="prP", bufs=1) as pr_pool, \
         tc.tile_pool(name="saP", bufs=1) as sa_pool, \
         tc.tile_pool(name="smpP", bufs=2) as smp_pool:
        accs = [pacc_pool.tile([P, HW], F32, tag=f"pa{m}", name=f"pa{m}")
                for m in range(NM)]
        widx_v = widx[:].rearrange("p (t s) -> p t s", s=64)
        for t in range(NT):
            for h in range(2):
                pw = pw_pool.tile([P, HWD], BF16, tag="pw", name="pw")
                for (p0, p1, g) in G_SPANS:
                    nc.sync.dma_start(
                        pw[p0:p1, :],
                        wc_dram[gk_row(g, t):gk_row(g, t) + 1,
                                h * HWD:(h + 1) * HWD].broadcast_to(
                            [p1 - p0, HWD]),
                    )
                gA = ga_pool.tile([P, 512, 12], BF16, tag="gA", name="gA")
                nc.gpsimd.ap_gather(gA[:], d3[:], widx_v[:, t, h * 32:(h + 1) * 32],
                                    channels=P, num_elems=PHW, d=12, num_idxs=512)
                prod = pr_pool.tile([P, HWD], BF16, tag="prod", name="prod")
                nc.vector.tensor_tensor(
                    prod[:], gA[:].rearrange("c m j -> c (m j)"), pw[:], op=ALU.mult)
                prv = prod[:].rearrange("c (m i j) -> c m i j", i=3, j=4)
                sa = sa_pool.tile([P, 512, 3, 2], BF16, tag="sa", name="sa")
                nc.vector.tensor_tensor(sa[:], prv[:, :, :, 0:2], prv[:, :, :, 2:4],
                                        op=ALU.add)
                smp = smp_pool.tile([P, 512, 3], BF16, tag="smp", name="smp")
                nc.vector.tensor_tensor(smp[:], sa[:, :, :, 0], sa[:, :, :, 1],
                                        op=ALU.add)
                for ich in range(3):
                    first = (t == 0 and ich == 0)
                    last = (t == NT - 1 and ich == 2)
                    for m in range(NM):
                        nc.tensor.matmul(
                            accs[m][:, h * 512:(h + 1) * 512],
                            w_dcn3_v[:, t * 3 + ich, m * P:(m + 1) * P],
                            smp[:, :, ich],
                            start=first, stop=last,
                        )
        # BN + SiLU in one activation per output tile
        for m in range(NM):
            nc.scalar.activation(z[m][:], accs[m][:], ACTF.Silu,
                                 bias=bn_shift[:, m:m + 1], scale=bn_scale[:, m:m + 1])

    # ---------- phase 8: 1x1 conv, pixel-major (output needs no transpose) ----------
    w2r_v = w2r[:].rearrange("c (k o) -> c k o", o=DIM)
    with tc.tile_pool(name="p8", bufs=3, space="PSUM") as p8_pool, \
         tc.tile_pool(name="osb", bufs=3) as osb_pool:
        for pt in range(NPT):
            ps = p8_pool.tile([P, DIM], F32)
            for kt in range(NCT):
                nc.tensor.matmul(
                    ps[:], z[kt][:, pt * P:(pt + 1) * P], w2r_v[:, kt, :],
                    start=(kt == 0), stop=False,
                )
            nc.tensor.matmul(ps[:], ones1[0:1, :], b2r[0:1, :],
                             start=False, stop=True)
            osb = osb_pool.tile([P, DIM], F32, tag="osb", name="osb")
            nc.scalar.activation(osb[:], ps[:], ACTF.Copy)
            nc.sync.dma_start(io["out"][pt * P:(pt + 1) * P, :], osb[:])


# ======================================================================
# SPMD entry point: full inputs in, full output out (8 cores, batch-parallel)
# ======================================================================

_PROGRAM_CACHE = {}


def _get_program(consts):
    key = "dcn2"
    if key not in _PROGRAM_CACHE:
        import concourse.bacc as bacc
        nc = bacc.Bacc("TRN2", target_bir_lowering=False, debug=False)
        io = declare_io(nc, consts)
        with tile.TileContext(nc) as tc:
            with ExitStack() as ctx:
                build(ctx, tc, io)
        nc.compile()
        _PROGRAM_CACHE[key] = nc
    return _PROGRAM_CACHE[key]


def kernel(**inputs) -> np.ndarray:
    from concourse.bass_utils import run_bass_kernel_spmd

    x = np.ascontiguousarray(np.asarray(inputs["x"], np.float32))
    B = x.shape[0]
    assert x.shape == (B, HW, DIM), x.shape
    consts = host_prep(inputs)
    nc = _get_program(consts)
    n_cores = 8
    reps = []
    for i in range(n_cores):
        m = {"x": x[i % B]}
        m.update(consts)
        reps.append(m)
    res = run_bass_kernel_spmd(nc, reps, list(range(n_cores)))
    out = np.stack([np.asarray(res.results[i]["out"], np.float32)
                    for i in range(B)], axis=0)
    return out


# revision 14
# speedup vs baseline: 1.4879x; 1.1530x over previous
import os as _os
import sys as _sys

for _p in ("/opt/trn_rl_repo", "/root/.axon_site/_ro/trn_rl_repo",
           "/root/.axon_site", "/root/.axon_site/_ro/pypackages"):
    if _os.path.isdir(_p) and _p not in _sys.path:
        _sys.path.append(_p)

"""DCNv2 block kernel for TRN2 (Bass/Tile), v2.

Per-core program: one batch sample.
  x [1024, 384] -> transpose -> padded bf16 image xtp [384ch, 42*42]
  offset conv 3x3 (384->72, bf16) -> positions -> floor/frac/corner weights
  corner weights broadcast to channel partitions via DRAM-bounce DMA (bf16)
  ap_gather (d=4 packed corners, bf16) -> one TT mult + windowed reduce
  dcn matmul (K=3456, bf16) -> BN+SiLU (one Silu activation) -> 1x1 conv in
  pixel-major form (z as lhsT) -> out [1024, 384] with no output transposes.
"""

import numpy as np
from contextlib import ExitStack

import concourse.bass as bass
import concourse.tile as tile
from concourse import mybir
from concourse import library_config

F32 = mybir.dt.float32
BF16 = mybir.dt.bfloat16
I16 = mybir.dt.int16
I32 = mybir.dt.int32
ALU = mybir.AluOpType
ACTF = mybir.ActivationFunctionType

DIM, KK, G, Cg = 384, 9, 4, 96
H = W = 32
HW = 1024
PAD = 4
PH = PW = H + 2 * PAD          # 40
PHW = PH * PW                  # 1600
NT = KK                        # 9 taps
NCT = DIM // 128               # 3
NM = DIM // 128                # 3
OFFP = 100                     # offset rows: dy 0..35, dx 64..99
XOFF = 64
NPT = HW // 128                # 8
MAGIC = float(2 ** 23)

# (start, end, group) partition spans per channel tile
CT_SPANS = [
    [(0, 96, 0), (96, 128, 1)],
    [(0, 64, 1), (64, 128, 2)],
    [(0, 32, 2), (32, 128, 3)],
]


def gk_row(g, k):
    return g * KK + k


def host_prep(inputs: dict) -> dict:
    """Pure-layout host prep of weights/constants (shared by all cores)."""
    import ml_dtypes
    w_off = np.asarray(inputs["w_off"], np.float32)      # [72, 384, 3, 3]
    b_off = np.asarray(inputs["b_off"], np.float32)      # [72]
    w_dcn = np.asarray(inputs["w_dcn"], np.float32)      # [384, 384, 3, 3]
    w2 = np.asarray(inputs["w2"], np.float32)            # [384, 384]

    # offset conv rows: gk = dy rows 0..35, 36+gk = dx rows
    w_off_p = np.zeros((OFFP, DIM, 3, 3), np.float32)
    b_off_p = np.zeros((36, 2), np.float32)
    for g in range(G):
        for k in range(KK):
            w_off_p[gk_row(g, k)] = w_off[g * 18 + k * 2 + 0]
            w_off_p[XOFF + gk_row(g, k)] = w_off[g * 18 + k * 2 + 1]
            b_off_p[gk_row(g, k), 0] = b_off[g * 18 + k * 2 + 0]
            b_off_p[gk_row(g, k), 1] = b_off[g * 18 + k * 2 + 1]

    # offset conv lhsT tiles [128, 27, 72] bf16; K order = (tap, ct)
    w_offT = np.zeros((128, NT * NCT, OFFP), np.float32)
    for t in range(NT):
        ky, kx = t // 3, t % 3
        for ct in range(NCT):
            cs = ct * 128
            w_offT[:, t * NCT + ct, :] = w_off_p[:, cs:cs + 128, ky, kx].T
    w_offT = w_offT.reshape(128, NT * NCT * OFFP)

    # grid [36, 2*HW] f32, cols in rho order (rho(n) = (n%64)*16 + n//64)
    jj = np.arange(HW)
    rho = (jj % 64) * 16 + jj // 64
    yy = (np.arange(HW) // W)[rho]
    xx = (np.arange(HW) % W)[rho]
    grid_s = np.zeros((36, 2 * HW), np.float32)
    for g in range(G):
        for k in range(KK):
            grid_s[gk_row(g, k), 0:HW] = (k // 3 - 1) + yy + PAD
            grid_s[gk_row(g, k), HW:] = (k % 3 - 1) + xx + PAD

    # dcn lhsT tiles [128, 9*3, 384] bf16, rows follow the packed-channel
    # partition map: partition P holds channels (P//32)*96 + ich*32 + P%32.
    w_dcn_r = w_dcn.reshape(DIM, DIM, KK)
    PP = np.arange(128)
    w_dcn3 = np.zeros((128, NT * 3, DIM), np.float32)
    for t in range(NT):
        for ich in range(3):
            cmap = (PP // 32) * 96 + ich * 32 + (PP % 32)
            w_dcn3[:, t * 3 + ich, :] = w_dcn_r[:, cmap, t].T
    w_dcn3 = w_dcn3.reshape(128, NT * 3 * DIM)

    # 1x1 conv rhs tiles (pixel-major matmul): w2r[c, kt*384+o] = w2[o, kt*128+c]
    w2r = np.zeros((128, NCT, DIM), np.float32)
    for kt in range(NCT):
        w2r[:, kt, :] = w2[:, kt * 128:(kt + 1) * 128].T
    w2r = w2r.reshape(128, NCT * DIM)

    consts = {
        "w_offT": w_offT.astype(ml_dtypes.bfloat16),
        "b_off_p": b_off_p,
        "grid_s": grid_s,
        "w_dcn3": w_dcn3.astype(ml_dtypes.bfloat16),
        "w2r": w2r.astype(ml_dtypes.bfloat16),
        "b2r": np.asarray(inputs["b2"], np.float32).reshape(1, DIM).astype(ml_dtypes.bfloat16),
        "ones1": np.ones((1, 128), np.float32).astype(ml_dtypes.bfloat16),
        "ident": np.eye(128, dtype=np.float32),
        "ident_bf": np.eye(128, dtype=np.float32).astype(ml_dtypes.bfloat16),
        "sconst": np.tile(np.array([[MAGIC, -MAGIC, float(PW), 1.0, -1.0]],
                                   np.float32), (36, 1)),
        "bn_gamma": np.asarray(inputs["bn_gamma"], np.float32),
        "bn_beta": np.asarray(inputs["bn_beta"], np.float32),
        "bn_mean": np.asarray(inputs["bn_mean"], np.float32),
        "bn_var": np.asarray(inputs["bn_var"], np.float32),
    }
    return consts


def declare_io(nc: bass.Bass, consts: dict):
    aps = {}
    aps["x"] = nc.dram_tensor("x", [HW, DIM], F32, kind="ExternalInput").ap()
    for name, arr in consts.items():
        dt = {np.dtype("float32"): F32}.get(arr.dtype, BF16)
        aps[name] = nc.dram_tensor(name, list(arr.shape), dt, kind="ExternalInput").ap()
    aps["out"] = nc.dram_tensor("out", [HW, DIM], F32, kind="ExternalOutput").ap()
    return aps


def build(ctx: ExitStack, tc: tile.TileContext, io: dict):
    nc = tc.nc
    P = 128
    nc.gpsimd.load_library(library_config.ap_gather)

    const_pool = ctx.enter_context(tc.tile_pool(name="consts", bufs=1))
    d2_pool = ctx.enter_context(tc.tile_pool(name="d2", bufs=1))
    mid_pool = ctx.enter_context(tc.tile_pool(name="mid", bufs=1))

    # ---------- constants ----------
    w_offT = const_pool.tile([P, NT * NCT * OFFP], BF16)
    nc.sync.dma_start(w_offT[:], io["w_offT"][:])
    grid_s = const_pool.tile([36, 2 * HW], F32)
    nc.sync.dma_start(grid_s[:], io["grid_s"][:])
    w_dcn3 = const_pool.tile([P, NT * 3 * DIM], BF16)
    nc.sync.dma_start(w_dcn3[:], io["w_dcn3"][:])
    w2r = const_pool.tile([P, NCT * DIM], BF16)
    nc.sync.dma_start(w2r[:], io["w2r"][:])
    b2r = const_pool.tile([1, DIM], BF16)
    nc.sync.dma_start(b2r[:], io["b2r"][:])
    ones1 = const_pool.tile([1, P], BF16)
    nc.sync.dma_start(ones1[:], io["ones1"][:])
    ident = const_pool.tile([P, P], F32)
    nc.sync.dma_start(ident[:], io["ident"][:])
    identb = const_pool.tile([P, P], BF16)
    nc.sync.dma_start(identb[:], io["ident_bf"][:])
    b_off_s = const_pool.tile([36, 2], F32)
    nc.sync.dma_start(b_off_s[:], io["b_off_p"][:])
    sconst = const_pool.tile([36, 5], F32)
    nc.sync.dma_start(sconst[:], io["sconst"][:])

    bnv = {}
    for vname in ("bn_gamma", "bn_beta", "bn_mean", "bn_var"):
        tl = const_pool.tile([P, NM], F32, tag=f"bn_{vname}", name=f"bn_{vname}")
        for m in range(NM):
            nc.sync.dma_start(
                tl[:, m:m + 1],
                io[vname][:].rearrange("(m p u) -> m p u", p=P, u=1)[m],
            )
        bnv[vname] = tl
    bn_scale = const_pool.tile([P, NM], F32)
    bn_shift = const_pool.tile([P, NM], F32)
    tmpv = const_pool.tile([P, NM], F32)
    nc.vector.tensor_scalar(tmpv[:], bnv["bn_var"][:], 1e-5, None, op0=ALU.add)
    nc.scalar.sqrt(tmpv[:], tmpv[:])
    nc.vector.reciprocal(tmpv[:], tmpv[:])
    nc.vector.tensor_tensor(bn_scale[:], bnv["bn_gamma"][:], tmpv[:], op=ALU.mult)
    nc.vector.tensor_tensor(tmpv[:], bnv["bn_mean"][:], bn_scale[:], op=ALU.mult)
    nc.vector.tensor_tensor(bn_shift[:], bnv["bn_beta"][:], tmpv[:], op=ALU.subtract)

    # ---------- phase 1: load x, transpose into padded bf16 image ----------
    xtp_cm = tc.tile_pool(name="xtp", bufs=1)
    xtp_pool = xtp_cm.__enter__()
    XTW = PHW + PW + 2
    xtp = [xtp_pool.tile([P, XTW], BF16, tag=f"xtp{ct}", name=f"xtp{ct}") for ct in range(NCT)]
    for ct in range(NCT):
        nc.vector.memset(xtp[ct][:], 0.0)

    with tc.tile_pool(name="ptrans", bufs=4, space="PSUM") as psum_t, \
         tc.tile_pool(name="xin", bufs=3) as xin_pool:
        for pt in range(NPT):
            xin = xin_pool.tile([P, DIM], F32)
            nc.sync.dma_start(xin[:], io["x"][pt * P:(pt + 1) * P, :])
            for ct in range(NCT):
                ps = psum_t.tile([P, P], F32)
                nc.tensor.transpose(ps[:], xin[:, ct * P:(ct + 1) * P], ident[:])
                dst = xtp[ct][:, 0:PHW].rearrange("c (y x) -> c y x", x=PW)
                dst = dst[:, PAD + pt * 4: PAD + pt * 4 + 4, PAD:PAD + W]
                nc.scalar.activation(dst, ps[:].rearrange("c (r j) -> c r j", j=W),
                                     ACTF.Copy)

    # ---------- phase 3: offset conv (bf16) ----------
    off_s = mid_pool.tile([36, 2 * HW], F32, name="off_s")
    with tc.tile_pool(name="poff", bufs=1, space="PSUM") as poff_pool:
        ps_off = poff_pool.tile([OFFP, HW], F32)
        w_offT_v = w_offT[:].rearrange("r (k o) -> r k o", o=OFFP)
        n_k = NT * NCT
        for t in range(NT):
            ky, kx = t // 3, t % 3
            for ct in range(NCT):
                kt = t * NCT + ct
                rhs = xtp[ct][:, 0:PHW].rearrange("c (y x) -> c y x", x=PW)
                rhs = rhs[:, PAD - 1 + ky:PAD - 1 + ky + H, PAD - 1 + kx:PAD - 1 + kx + W]
                rhs = rhs.rearrange("c y (xh p) -> c p y xh", p=16)
                for nh in range(2):
                    nc.tensor.matmul(
                        ps_off[:, nh * 512:(nh + 1) * 512],
                        w_offT_v[:, kt, :],
                        rhs[:, nh * 8:(nh + 1) * 8, :, :],
                        start=(kt == 0), stop=(kt == n_k - 1),
                    )
        nc.scalar.activation(off_s[:, 0:HW], ps_off[0:36, :], ACTF.Identity,
                             bias=b_off_s[:, 0:1])
        nc.scalar.activation(off_s[:, HW:], ps_off[XOFF:XOFF + 36, :], ACTF.Identity,
                             bias=b_off_s[:, 1:2])

    # ---------- phase 2: channel-packed 4-corner bf16 image ----------
    # d3[P, q, ich*4+j] = x[(P//32)*96 + ich*32 + P%32, q + sh_j]; one gather
    # per tap then serves all 384 channels (3 per partition).
    d3 = d2_pool.tile([P, PHW, 12], BF16, name="d3")
    CW = 469
    SHJ = (0, 1, PW, PW + 1)
    with tc.tile_pool(name="p3b", bufs=3, space="PSUM") as p3b_pool:
        for ich in range(3):
            for ck in range(4):
                q0 = ck * CW
                cw = min(CW, PHW - q0)
                psA = p3b_pool.tile([96, 512], F32, tag="psA", name="psA")
                psB = p3b_pool.tile([32, 512], F32, tag="psB", name="psB")
                for g in range(G):
                    c0 = g * 96 + ich * 32
                    T, off = c0 // 128, c0 % 128
                    dst = (psA[g * 32:(g + 1) * 32, 0:cw + PW + 2] if g < 3
                           else psB[:, 0:cw + PW + 2])
                    nc.tensor.matmul(
                        dst,
                        identb[:, off:off + 32],
                        xtp[T][:, q0:q0 + cw + PW + 2],
                        start=True, stop=True,
                    )
                for j, sh in enumerate(SHJ):
                    if j % 2 == 0:
                        nc.scalar.activation(d3[0:96, q0:q0 + cw, ich * 4 + j],
                                             psA[:, sh:sh + cw], ACTF.Copy)
                        nc.vector.tensor_copy(d3[96:128, q0:q0 + cw, ich * 4 + j],
                                              psB[:, sh:sh + cw])
                    else:
                        nc.vector.tensor_copy(d3[0:96, q0:q0 + cw, ich * 4 + j],
                                              psA[:, sh:sh + cw])
                        nc.scalar.activation(d3[96:128, q0:q0 + cw, ich * 4 + j],
                                             psB[:, sh:sh + cw], ACTF.Copy)

    xtp_cm.__exit__(None, None, None)

    # ---------- phase 4: positions, indices ----------
    W2 = 2 * HW
    small_cm = tc.tile_pool(name="small", bufs=1)
    small_pool = small_cm.__enter__()
    pos = small_pool.tile([36, W2], F32, name="pos")
    nc.vector.tensor_tensor(pos[:], off_s[:], grid_s[:], op=ALU.add)
    rnd = small_pool.tile([36, W2], F32, name="rnd")
    nc.scalar.add(rnd[:], pos[:], sconst[:, 0:1])
    nc.scalar.add(rnd[:], rnd[:], sconst[:, 1:2])
    cmp = small_pool.tile([36, W2], F32, name="cmp")
    nc.vector.tensor_tensor(cmp[:], rnd[:], pos[:], op=ALU.is_gt)
    flr = small_pool.tile([36, W2], F32, name="flr")
    nc.vector.tensor_tensor(flr[:], rnd[:], cmp[:], op=ALU.subtract)
    nc.vector.tensor_scalar(flr[:, 0:HW], flr[:, 0:HW], 0.0, float(PH - 2),
                            op0=ALU.max, op1=ALU.min)
    nc.vector.tensor_scalar(flr[:, HW:], flr[:, HW:], 0.0, float(PW - 2),
                            op0=ALU.max, op1=ALU.min)

    # indices first (unblocks phase 5 / gathers early)
    qf = small_pool.tile([36, HW], F32, name="qf")
    nc.scalar.mul(qf[:], flr[:, 0:HW], sconst[:, 2:3])
    nc.vector.tensor_tensor(qf[:], qf[:], flr[:, HW:], op=ALU.add)
    qi32 = small_pool.tile([36, HW], I32, name="qi32")
    nc.vector.tensor_copy(qi32[:], qf[:])
    qi16 = small_pool.tile([36, HW], I16, name="qi16")
    nc.vector.tensor_copy(qi16[:], qi32[:])

    # ---------- phase 5a: wrap indices via DRAM bounce ----------
    dram_pool = ctx.enter_context(tc.tile_pool(name="qdram", bufs=1, space="DRAM"))
    qa_dram = dram_pool.tile([36, HW], I16, name="qa_dram")
    nc.sync.dma_start(qa_dram[:], qi16[:])
    widx = mid_pool.tile([P, NT * 64], I16, name="widx")
    for cb in range(8):
        g = cb // 2
        dst = widx[cb * 16:(cb + 1) * 16, :].rearrange("p (t s) -> p t s", s=64)
        srcv = qa_dram[gk_row(g, 0):gk_row(g, 0) + NT, :]
        srcv = srcv.rearrange("t (p s) -> p t s", p=16)
        nc.sync.dma_start(dst, srcv)

    # ---------- phase 4b: corner weights, packed in gather output order ----------
    frac = small_pool.tile([36, W2], F32, name="frac")
    nc.vector.tensor_tensor(frac[:], pos[:], flr[:], op=ALU.subtract)
    gyx = small_pool.tile([36, W2], F32, name="gyx")
    nc.scalar.activation(gyx[:], frac[:], ACTF.Identity,
                         bias=sconst[:, 3:4], scale=sconst[:, 4:5])
    # wc_packed[g, m, ich*4+j]: corner weights for gather-output col m
    # (m-order), replicated across the 3 packed channels; source cols are
    # n-ordered with n = (m%16)*64 + m//16.
    wc_packed = small_pool.tile([36, HW * 12], BF16, name="wc_packed")
    wcp_v = wc_packed[:].rearrange("g (r s i j) -> g s r i j", r=64, s=16, i=3, j=4)
    fy = frac[:, 0:HW].rearrange("g (s r) -> g s r", s=16)
    fx = frac[:, HW:].rearrange("g (s r) -> g s r", s=16)
    gy = gyx[:, 0:HW].rearrange("g (s r) -> g s r", s=16)
    gx = gyx[:, HW:].rearrange("g (s r) -> g s r", s=16)
    for i in range(3):
        nc.vector.tensor_tensor(wcp_v[:, :, :, i, 0], gy, gx, op=ALU.mult)
        nc.vector.tensor_tensor(wcp_v[:, :, :, i, 1], gy, fx, op=ALU.mult)
        nc.vector.tensor_tensor(wcp_v[:, :, :, i, 2], fy, gx, op=ALU.mult)
        nc.vector.tensor_tensor(wcp_v[:, :, :, i, 3], fy, fx, op=ALU.mult)

    # ---------- phase 5b: weight table to DRAM for broadcast ----------
    wc_dram = dram_pool.tile([36, HW * 12], BF16, name="wc_dram")
    nc.sync.dma_start(wc_dram[:], wc_packed[:])
    small_cm.__exit__(None, None, None)

    # ---------- phase 6+7: gather, weight, reduce, dcn matmul ----------
    # One packed gather per (tap, pixel-half) serves all 384 channels.
    w_dcn3_v = w_dcn3[:].rearrange("r (k o) -> r k o", o=DIM)
    z = [mid_pool.tile([P, HW], BF16, tag=f"z{m}", name=f"z{m}") for m in range(NM)]
    G_SPANS = [(0, 32, 0), (32, 64, 1), (64, 96, 2), (96, 128, 3)]
    HWD = 512 * 12
    with tc.tile_pool(name="pacc", bufs=1, space="PSUM") as pacc_pool, \
         tc.tile_pool(name="gaP", bufs=2) as ga_pool, \
         tc.tile_pool(name="pwP", bufs=2) as pw_pool, \
         tc.tile_pool(name="prP", bufs=1) as pr_pool, \
         tc.tile_pool(name="saP", bufs=1) as sa_pool, \
         tc.tile_pool(name="smpP", bufs=2) as smp_pool:
        accs = [pacc_pool.tile([P, HW], F32, tag=f"pa{m}", name=f"pa{m}")
                for m in range(NM)]
        widx_v = widx[:].rearrange("p (t s) -> p t s", s=64)
        for t in range(NT):
            for h in range(2):
                pw = pw_pool.tile([P, HWD], BF16, tag="pw", name="pw")
                for (p0, p1, g) in G_SPANS:
                    nc.sync.dma_start(
                        pw[p0:p1, :],
                        wc_dram[gk_row(g, t):gk_row(g, t) + 1,
                                h * HWD:(h + 1) * HWD].broadcast_to(
                            [p1 - p0, HWD]),
                    )
                gA = ga_pool.tile([P, 512, 12], BF16, tag="gA", name="gA")
                nc.gpsimd.ap_gather(gA[:], d3[:], widx_v[:, t, h * 32:(h + 1) * 32],
                                    channels=P, num_elems=PHW, d=12, num_idxs=512)
                prod = pr_pool.tile([P, HWD], BF16, tag="prod", name="prod")
                nc.vector.tensor_tensor(
                    prod[:], gA[:].rearrange("c m j -> c (m j)"), pw[:], op=ALU.mult)
                prv = prod[:].rearrange("c (m i j) -> c m i j", i=3, j=4)
                sa = sa_pool.tile([P, 512, 3, 2], BF16, tag="sa", name="sa")
                nc.vector.tensor_tensor(sa[:], prv[:, :, :, 0:2], prv[:, :, :, 2:4],
                                        op=ALU.add)
                smp = smp_pool.tile([P, 512, 3], BF16, tag="smp", name="smp")
                nc.vector.tensor_tensor(smp[:], sa[:, :, :, 0], sa[:, :, :, 1],
                                        op=ALU.add)
                for ich in range(3):
                    first = (t == 0 and ich == 0)
                    last = (t == NT - 1 and ich == 2)
                    for m in range(NM):
                        nc.tensor.matmul(
                            accs[m][:, h * 512:(h + 1) * 512],
                            w_dcn3_v[:, t * 3 + ich, m * P:(m + 1) * P],
                            smp[:, :, ich],
                            start=first, stop=last,
                        )
        # BN + SiLU in one activation per output tile
        for m in range(NM):
            nc.scalar.activation(z[m][:], accs[m][:], ACTF.Silu,
                                 bias=bn_shift[:, m:m + 1], scale=bn_scale[:, m:m + 1])

    # ---------- phase 8: 1x1 conv, pixel-major (output needs no transpose) ----------
    w2r_v = w2r[:].rearrange("c (k o) -> c k o", o=DIM)
    with tc.tile_pool(name="p8", bufs=3, space="PSUM") as p8_pool, \
         tc.tile_pool(name="osb", bufs=3) as osb_pool:
        for pt in range(NPT):
            ps = p8_pool.tile([P, DIM], F32)
            for kt in range(NCT):
                nc.tensor.matmul(
                    ps[:], z[kt][:, pt * P:(pt + 1) * P], w2r_v[:, kt, :],
                    start=(kt == 0), stop=False,
                )
            nc.tensor.matmul(ps[:], ones1[0:1, :], b2r[0:1, :],
                             start=False, stop=True)
            osb = osb_pool.tile([P, DIM], F32, tag="osb", name="osb")
            nc.scalar.activation(osb[:], ps[:], ACTF.Copy)
            nc.sync.dma_start(io["out"][pt * P:(pt + 1) * P, :], osb[:])


# ======================================================================
# SPMD entry point: full inputs in, full output out (8 cores, batch-parallel)
# ======================================================================

_PROGRAM_CACHE = {}


def _get_program(consts):
    key = "dcn2"
    if key not in _PROGRAM_CACHE:
        import concourse.bacc as bacc
        nc = bacc.Bacc("TRN2", target_bir_lowering=False, debug=False)
        io = declare_io(nc, consts)
        with tile.TileContext(nc) as tc:
            with ExitStack() as ctx:
                build(ctx, tc, io)
        nc.compile()
        _PROGRAM_CACHE[key] = nc
    return _PROGRAM_CACHE[key]


def kernel(**inputs) -> np.ndarray:
    from concourse.bass_utils import run_bass_kernel_spmd

    x = np.ascontiguousarray(np.asarray(inputs["x"], np.float32))
    B = x.shape[0]
    assert x.shape == (B, HW, DIM), x.shape
    consts = host_prep(inputs)
    nc = _get_program(consts)
    n_cores = 8
    reps = []
    for i in range(n_cores):
        m = {"x": x[i % B]}
        m.update(consts)
        reps.append(m)
    res = run_bass_kernel_spmd(nc, reps, list(range(n_cores)))
    out = np.stack([np.asarray(res.results[i]["out"], np.float32)
                    for i in range(B)], axis=0)
    return out


# revision 15
# speedup vs baseline: 1.6514x; 1.1099x over previous
import os as _os
import sys as _sys

for _p in ("/opt/trn_rl_repo", "/root/.axon_site/_ro/trn_rl_repo",
           "/root/.axon_site", "/root/.axon_site/_ro/pypackages"):
    if _os.path.isdir(_p) and _p not in _sys.path:
        _sys.path.append(_p)

"""DCNv2 block kernel for TRN2 (Bass/Tile), v2.

Per-core program: one batch sample.
  x [1024, 384] -> transpose -> padded bf16 image xtp [384ch, 42*42]
  offset conv 3x3 (384->72, bf16) -> positions -> floor/frac/corner weights
  corner weights broadcast to channel partitions via DRAM-bounce DMA (bf16)
  ap_gather (d=4 packed corners, bf16) -> one TT mult + windowed reduce
  dcn matmul (K=3456, bf16) -> BN+SiLU (one Silu activation) -> 1x1 conv in
  pixel-major form (z as lhsT) -> out [1024, 384] with no output transposes.
"""

import copy as _copy
import numpy as np
from contextlib import ExitStack

import concourse.bass as bass
import concourse.tile as tile
from concourse import mybir
from concourse import library_config

F32 = mybir.dt.float32
BF16 = mybir.dt.bfloat16
I16 = mybir.dt.int16
I32 = mybir.dt.int32
ALU = mybir.AluOpType
ACTF = mybir.ActivationFunctionType

DIM, KK, G, Cg = 384, 9, 4, 96
H = W = 32
HW = 1024
PAD = 4
PH = PW = H + 2 * PAD          # 40
PHW = PH * PW                  # 1600
NT = KK                        # 9 taps
NCT = DIM // 128               # 3
NM = DIM // 128                # 3
OFFP = 100                     # offset rows: dy 0..35, dx 64..99
XOFF = 64
NPT = HW // 128                # 8
MAGIC = float(2 ** 23)

# (start, end, group) partition spans per channel tile
CT_SPANS = [
    [(0, 96, 0), (96, 128, 1)],
    [(0, 64, 1), (64, 128, 2)],
    [(0, 32, 2), (32, 128, 3)],
]


def gk_row(g, k):
    return g * KK + k


def host_prep(inputs: dict) -> dict:
    """Pure-layout host prep of weights/constants (shared by all cores)."""
    import ml_dtypes
    w_off = np.asarray(inputs["w_off"], np.float32)      # [72, 384, 3, 3]
    b_off = np.asarray(inputs["b_off"], np.float32)      # [72]
    w_dcn = np.asarray(inputs["w_dcn"], np.float32)      # [384, 384, 3, 3]
    w2 = np.asarray(inputs["w2"], np.float32)            # [384, 384]

    # offset conv rows: gk = dy rows 0..35, 36+gk = dx rows
    w_off_p = np.zeros((OFFP, DIM, 3, 3), np.float32)
    b_off_p = np.zeros((36, 2), np.float32)
    for g in range(G):
        for k in range(KK):
            w_off_p[gk_row(g, k)] = w_off[g * 18 + k * 2 + 0]
            w_off_p[XOFF + gk_row(g, k)] = w_off[g * 18 + k * 2 + 1]
            b_off_p[gk_row(g, k), 0] = b_off[g * 18 + k * 2 + 0]
            b_off_p[gk_row(g, k), 1] = b_off[g * 18 + k * 2 + 1]

    # offset conv lhsT tiles [128, 27, 72] bf16; K order = (tap, ct)
    w_offT = np.zeros((128, NT * NCT, OFFP), np.float32)
    for t in range(NT):
        ky, kx = t // 3, t % 3
        for ct in range(NCT):
            cs = ct * 128
            w_offT[:, t * NCT + ct, :] = w_off_p[:, cs:cs + 128, ky, kx].T
    w_offT = w_offT.reshape(128, NT * NCT * OFFP)

    # grid [36, 2*HW] f32, cols in rho order (rho(n) = (n%64)*16 + n//64)
    jj = np.arange(HW)
    rho = (jj % 64) * 16 + jj // 64
    yy = (np.arange(HW) // W)[rho]
    xx = (np.arange(HW) % W)[rho]
    grid_s = np.zeros((36, 2 * HW), np.float32)
    for g in range(G):
        for k in range(KK):
            grid_s[gk_row(g, k), 0:HW] = (k // 3 - 1) + yy + PAD
            grid_s[gk_row(g, k), HW:] = (k % 3 - 1) + xx + PAD

    # dcn lhsT tiles [128, 9*3, 384] bf16, rows follow the packed-channel
    # partition map: partition P holds channels (P//32)*96 + ich*32 + P%32.
    w_dcn_r = w_dcn.reshape(DIM, DIM, KK)
    PP = np.arange(128)
    w_dcn3 = np.zeros((128, NT * 3, DIM), np.float32)
    for t in range(NT):
        for ich in range(3):
            cmap = (PP // 32) * 96 + ich * 32 + (PP % 32)
            w_dcn3[:, t * 3 + ich, :] = w_dcn_r[:, cmap, t].T
    w_dcn3 = w_dcn3.reshape(128, NT * 3 * DIM)

    # 1x1 conv rhs tiles (pixel-major matmul): w2r[c, kt*384+o] = w2[o, kt*128+c]
    w2r = np.zeros((128, NCT, DIM), np.float32)
    for kt in range(NCT):
        w2r[:, kt, :] = w2[:, kt * 128:(kt + 1) * 128].T
    w2r = w2r.reshape(128, NCT * DIM)

    consts = {
        "w_offT": w_offT.astype(ml_dtypes.bfloat16),
        "b_off_p": b_off_p,
        "grid_s": grid_s,
        "w_dcn3": w_dcn3.astype(ml_dtypes.bfloat16),
        "w2r": w2r.astype(ml_dtypes.bfloat16),
        "b2r": np.asarray(inputs["b2"], np.float32).reshape(1, DIM).astype(ml_dtypes.bfloat16),
        "ones1": np.ones((1, 128), np.float32).astype(ml_dtypes.bfloat16),
        "ident": np.eye(128, dtype=np.float32),
        "ident_bf": np.eye(128, dtype=np.float32).astype(ml_dtypes.bfloat16),
        "sconst": np.tile(np.array([[MAGIC, -MAGIC, float(PW), 1.0, -1.0]],
                                   np.float32), (36, 1)),
        "bn_gamma": np.asarray(inputs["bn_gamma"], np.float32),
        "bn_beta": np.asarray(inputs["bn_beta"], np.float32),
        "bn_mean": np.asarray(inputs["bn_mean"], np.float32),
        "bn_var": np.asarray(inputs["bn_var"], np.float32),
    }
    return consts


def declare_io(nc: bass.Bass, consts: dict):
    aps = {}
    aps["x"] = nc.dram_tensor("x", [HW, DIM], F32, kind="ExternalInput").ap()
    for name, arr in consts.items():
        dt = {np.dtype("float32"): F32}.get(arr.dtype, BF16)
        aps[name] = nc.dram_tensor(name, list(arr.shape), dt, kind="ExternalInput").ap()
    aps["out"] = nc.dram_tensor("out", [HW, DIM], F32, kind="ExternalOutput").ap()
    return aps


def build(ctx: ExitStack, tc: tile.TileContext, io: dict):
    nc = tc.nc
    P = 128
    nc.gpsimd.load_library(library_config.ap_gather)

    const_pool = ctx.enter_context(tc.tile_pool(name="consts", bufs=1))
    d2_pool = ctx.enter_context(tc.tile_pool(name="d2", bufs=1))
    mid_pool = ctx.enter_context(tc.tile_pool(name="mid", bufs=1))

    # ---------- constants ----------
    w_offT = const_pool.tile([P, NT * NCT * OFFP], BF16)
    nc.sync.dma_start(w_offT[:], io["w_offT"][:])
    grid_s = const_pool.tile([36, 2 * HW], F32)
    nc.sync.dma_start(grid_s[:], io["grid_s"][:])
    w_dcn3 = const_pool.tile([P, NT * 3 * DIM], BF16)
    nc.sync.dma_start(w_dcn3[:], io["w_dcn3"][:])
    w2r = const_pool.tile([P, NCT * DIM], BF16)
    nc.sync.dma_start(w2r[:], io["w2r"][:])
    b2r = const_pool.tile([1, DIM], BF16)
    nc.sync.dma_start(b2r[:], io["b2r"][:])
    ones1 = const_pool.tile([1, P], BF16)
    nc.sync.dma_start(ones1[:], io["ones1"][:])
    ident = const_pool.tile([P, P], F32)
    nc.sync.dma_start(ident[:], io["ident"][:])
    identb = const_pool.tile([P, P], BF16)
    nc.sync.dma_start(identb[:], io["ident_bf"][:])
    b_off_s = const_pool.tile([36, 2], F32)
    nc.sync.dma_start(b_off_s[:], io["b_off_p"][:])
    sconst = const_pool.tile([36, 5], F32)
    nc.sync.dma_start(sconst[:], io["sconst"][:])

    bnv = {}
    for vname in ("bn_gamma", "bn_beta", "bn_mean", "bn_var"):
        tl = const_pool.tile([P, NM], F32, tag=f"bn_{vname}", name=f"bn_{vname}")
        for m in range(NM):
            nc.sync.dma_start(
                tl[:, m:m + 1],
                io[vname][:].rearrange("(m p u) -> m p u", p=P, u=1)[m],
            )
        bnv[vname] = tl
    bn_scale = const_pool.tile([P, NM], F32)
    bn_shift = const_pool.tile([P, NM], F32)
    tmpv = const_pool.tile([P, NM], F32)
    nc.vector.tensor_scalar(tmpv[:], bnv["bn_var"][:], 1e-5, None, op0=ALU.add)
    nc.scalar.sqrt(tmpv[:], tmpv[:])
    nc.vector.reciprocal(tmpv[:], tmpv[:])
    nc.vector.tensor_tensor(bn_scale[:], bnv["bn_gamma"][:], tmpv[:], op=ALU.mult)
    nc.vector.tensor_tensor(tmpv[:], bnv["bn_mean"][:], bn_scale[:], op=ALU.mult)
    nc.vector.tensor_tensor(bn_shift[:], bnv["bn_beta"][:], tmpv[:], op=ALU.subtract)

    # ---------- phase 1: load x, transpose into padded bf16 image ----------
    xtp_cm = tc.tile_pool(name="xtp", bufs=1)
    xtp_pool = xtp_cm.__enter__()
    XTW = PHW + PW + 2
    xtp = [xtp_pool.tile([P, XTW], BF16, tag=f"xtp{ct}", name=f"xtp{ct}") for ct in range(NCT)]
    for ct in range(NCT):
        nc.vector.memset(xtp[ct][:], 0.0)

    with tc.tile_pool(name="ptrans", bufs=4, space="PSUM") as psum_t, \
         tc.tile_pool(name="xin", bufs=3) as xin_pool:
        for pt in range(NPT):
            xin = xin_pool.tile([P, DIM], F32)
            nc.sync.dma_start(xin[:], io["x"][pt * P:(pt + 1) * P, :])
            for ct in range(NCT):
                ps = psum_t.tile([P, P], F32)
                nc.tensor.transpose(ps[:], xin[:, ct * P:(ct + 1) * P], ident[:])
                dst = xtp[ct][:, 0:PHW].rearrange("c (y x) -> c y x", x=PW)
                dst = dst[:, PAD + pt * 4: PAD + pt * 4 + 4, PAD:PAD + W]
                nc.scalar.activation(dst, ps[:].rearrange("c (r j) -> c r j", j=W),
                                     ACTF.Copy)

    # ---------- phase 3: offset conv (bf16) ----------
    off_s = mid_pool.tile([36, 2 * HW], F32, name="off_s")
    with tc.tile_pool(name="poff", bufs=1, space="PSUM") as poff_pool:
        ps_off = poff_pool.tile([OFFP, HW], F32)
        w_offT_v = w_offT[:].rearrange("r (k o) -> r k o", o=OFFP)
        n_k = NT * NCT
        for t in range(NT):
            ky, kx = t // 3, t % 3
            for ct in range(NCT):
                kt = t * NCT + ct
                rhs = xtp[ct][:, 0:PHW].rearrange("c (y x) -> c y x", x=PW)
                rhs = rhs[:, PAD - 1 + ky:PAD - 1 + ky + H, PAD - 1 + kx:PAD - 1 + kx + W]
                rhs = rhs.rearrange("c y (xh p) -> c p y xh", p=16)
                for nh in range(2):
                    nc.tensor.matmul(
                        ps_off[:, nh * 512:(nh + 1) * 512],
                        w_offT_v[:, kt, :],
                        rhs[:, nh * 8:(nh + 1) * 8, :, :],
                        start=(kt == 0), stop=(kt == n_k - 1),
                    )
        nc.scalar.activation(off_s[:, 0:HW], ps_off[0:36, :], ACTF.Identity,
                             bias=b_off_s[:, 0:1])
        nc.scalar.activation(off_s[:, HW:], ps_off[XOFF:XOFF + 36, :], ACTF.Identity,
                             bias=b_off_s[:, 1:2])

    # ---------- phase 2: channel-packed 4-corner bf16 image ----------
    # d3[P, q, ich*4+j] = x[(P//32)*96 + ich*32 + P%32, q + sh_j]; one gather
    # per tap then serves all 384 channels (3 per partition).
    d3 = d2_pool.tile([P, PHW, 12], BF16, name="d3")
    CW = 469
    SHJ = (0, 1, PW, PW + 1)
    with tc.tile_pool(name="p3b", bufs=3, space="PSUM") as p3b_pool:
        for ich in range(3):
            for ck in range(4):
                q0 = ck * CW
                cw = min(CW, PHW - q0)
                psA = p3b_pool.tile([96, 512], F32, tag="psA", name="psA")
                psB = p3b_pool.tile([32, 512], F32, tag="psB", name="psB")
                for g in range(G):
                    c0 = g * 96 + ich * 32
                    T, off = c0 // 128, c0 % 128
                    dst = (psA[g * 32:(g + 1) * 32, 0:cw + PW + 2] if g < 3
                           else psB[:, 0:cw + PW + 2])
                    nc.tensor.matmul(
                        dst,
                        identb[:, off:off + 32],
                        xtp[T][:, q0:q0 + cw + PW + 2],
                        start=True, stop=True,
                    )
                def _ovl(base_ap, cw=cw):
                    ap2 = _copy.copy(base_ap)
                    ap2.ap = mybir.VecI64Pair(
                        [list(base_ap.ap[0]), [1, cw], [PW, 2], [1, 2]])
                    return ap2
                dstA = d3[0:96, q0:q0 + cw, ich * 4:ich * 4 + 4]
                dstB = d3[96:128, q0:q0 + cw, ich * 4:ich * 4 + 4]
                if (ich + ck) % 2 == 0:
                    nc.scalar.activation(dstA, _ovl(psA[:]), ACTF.Copy)
                    nc.vector.tensor_copy(dstB, _ovl(psB[:]))
                else:
                    nc.vector.tensor_copy(dstA, _ovl(psA[:]))
                    nc.scalar.activation(dstB, _ovl(psB[:]), ACTF.Copy)
    xtp_cm.__exit__(None, None, None)

    # ---------- phase 4: positions, indices ----------
    W2 = 2 * HW
    small_cm = tc.tile_pool(name="small", bufs=1)
    small_pool = small_cm.__enter__()
    pos = small_pool.tile([36, W2], F32, name="pos")
    nc.vector.tensor_tensor(pos[:], off_s[:], grid_s[:], op=ALU.add)
    rnd = small_pool.tile([36, W2], F32, name="rnd")
    nc.scalar.add(rnd[:], pos[:], sconst[:, 0:1])
    nc.scalar.add(rnd[:], rnd[:], sconst[:, 1:2])
    cmp = small_pool.tile([36, W2], F32, name="cmp")
    nc.vector.tensor_tensor(cmp[:], rnd[:], pos[:], op=ALU.is_gt)
    flr = small_pool.tile([36, W2], F32, name="flr")
    nc.vector.tensor_tensor(flr[:], rnd[:], cmp[:], op=ALU.subtract)
    nc.vector.tensor_scalar(flr[:, 0:HW], flr[:, 0:HW], 0.0, float(PH - 2),
                            op0=ALU.max, op1=ALU.min)
    nc.vector.tensor_scalar(flr[:, HW:], flr[:, HW:], 0.0, float(PW - 2),
                            op0=ALU.max, op1=ALU.min)

    # indices first (unblocks phase 5 / gathers early)
    qf = small_pool.tile([36, HW], F32, name="qf")
    nc.scalar.mul(qf[:], flr[:, 0:HW], sconst[:, 2:3])
    nc.vector.tensor_tensor(qf[:], qf[:], flr[:, HW:], op=ALU.add)
    qi32 = small_pool.tile([36, HW], I32, name="qi32")
    nc.vector.tensor_copy(qi32[:], qf[:])
    qi16 = small_pool.tile([36, HW], I16, name="qi16")
    nc.vector.tensor_copy(qi16[:], qi32[:])

    # ---------- phase 5a: wrap indices via DRAM bounce ----------
    dram_pool = ctx.enter_context(tc.tile_pool(name="qdram", bufs=1, space="DRAM"))
    qa_dram = dram_pool.tile([36, HW], I16, name="qa_dram")
    nc.sync.dma_start(qa_dram[:], qi16[:])
    widx = mid_pool.tile([P, NT * 64], I16, name="widx")
    for cb in range(8):
        g = cb // 2
        dst = widx[cb * 16:(cb + 1) * 16, :].rearrange("p (t s) -> p t s", s=64)
        srcv = qa_dram[gk_row(g, 0):gk_row(g, 0) + NT, :]
        srcv = srcv.rearrange("t (p s) -> p t s", p=16)
        nc.sync.dma_start(dst, srcv)

    # ---------- phase 4b: corner weights, packed in gather output order ----------
    frac = small_pool.tile([36, W2], F32, name="frac")
    nc.vector.tensor_tensor(frac[:], pos[:], flr[:], op=ALU.subtract)
    gyx = small_pool.tile([36, W2], F32, name="gyx")
    nc.scalar.activation(gyx[:], frac[:], ACTF.Identity,
                         bias=sconst[:, 3:4], scale=sconst[:, 4:5])
    # wc_packed[g, m, ich*4+j]: corner weights for gather-output col m
    # (m-order), replicated across the 3 packed channels; source cols are
    # n-ordered with n = (m%16)*64 + m//16.
    wc_packed = small_pool.tile([36, HW * 12], BF16, name="wc_packed")
    wcp_v = wc_packed[:].rearrange("g (r s i j) -> g s r i j", r=64, s=16, i=3, j=4)
    fy = frac[:, 0:HW].rearrange("g (s r) -> g s r", s=16)
    fx = frac[:, HW:].rearrange("g (s r) -> g s r", s=16)
    gy = gyx[:, 0:HW].rearrange("g (s r) -> g s r", s=16)
    gx = gyx[:, HW:].rearrange("g (s r) -> g s r", s=16)
    for i in range(3):
        nc.vector.tensor_tensor(wcp_v[:, :, :, i, 0], gy, gx, op=ALU.mult)
        nc.vector.tensor_tensor(wcp_v[:, :, :, i, 1], gy, fx, op=ALU.mult)
        nc.vector.tensor_tensor(wcp_v[:, :, :, i, 2], fy, gx, op=ALU.mult)
        nc.vector.tensor_tensor(wcp_v[:, :, :, i, 3], fy, fx, op=ALU.mult)

    # ---------- phase 5b: weight table to DRAM for broadcast ----------
    wc_dram = dram_pool.tile([36, HW * 12], BF16, name="wc_dram")
    nc.sync.dma_start(wc_dram[:], wc_packed[:])
    small_cm.__exit__(None, None, None)

    # ---------- phase 6+7: gather, weight, reduce, dcn matmul ----------
    # One packed gather per (tap, pixel-half) serves all 384 channels.
    w_dcn3_v = w_dcn3[:].rearrange("r (k o) -> r k o", o=DIM)
    z = [mid_pool.tile([P, HW], BF16, tag=f"z{m}", name=f"z{m}") for m in range(NM)]
    G_SPANS = [(0, 32, 0), (32, 64, 1), (64, 96, 2), (96, 128, 3)]
    HWD = 512 * 12
    with tc.tile_pool(name="pacc", bufs=1, space="PSUM") as pacc_pool, \
         tc.tile_pool(name="gaP", bufs=2) as ga_pool, \
         tc.tile_pool(name="pwP", bufs=2) as pw_pool, \
         tc.tile_pool(name="prP", bufs=1) as pr_pool, \
         tc.tile_pool(name="saP", bufs=1) as sa_pool, \
         tc.tile_pool(name="smpP", bufs=2) as smp_pool:
        accs = [pacc_pool.tile([P, HW], F32, tag=f"pa{m}", name=f"pa{m}")
                for m in range(NM)]
        widx_v = widx[:].rearrange("p (t s) -> p t s", s=64)
        for t in range(NT):
            for h in range(2):
                pw = pw_pool.tile([P, HWD], BF16, tag="pw", name="pw")
                for (p0, p1, g) in G_SPANS:
                    nc.sync.dma_start(
                        pw[p0:p1, :],
                        wc_dram[gk_row(g, t):gk_row(g, t) + 1,
                                h * HWD:(h + 1) * HWD].broadcast_to(
                            [p1 - p0, HWD]),
                    )
                gA = ga_pool.tile([P, 512, 12], BF16, tag="gA", name="gA")
                nc.gpsimd.ap_gather(gA[:], d3[:], widx_v[:, t, h * 32:(h + 1) * 32],
                                    channels=P, num_elems=PHW, d=12, num_idxs=512)
                prod = pr_pool.tile([P, HWD], BF16, tag="prod", name="prod")
                nc.vector.tensor_tensor(
                    prod[:], gA[:].rearrange("c m j -> c (m j)"), pw[:], op=ALU.mult)
                prv = prod[:].rearrange("c (m i j) -> c m i j", i=3, j=4)
                sa = sa_pool.tile([P, 512, 3, 2], BF16, tag="sa", name="sa")
                nc.vector.tensor_tensor(sa[:], prv[:, :, :, 0:2], prv[:, :, :, 2:4],
                                        op=ALU.add)
                smp = smp_pool.tile([P, 512, 3], BF16, tag="smp", name="smp")
                nc.vector.tensor_tensor(smp[:], sa[:, :, :, 0], sa[:, :, :, 1],
                                        op=ALU.add)
                for ich in range(3):
                    first = (t == 0 and ich == 0)
                    last = (t == NT - 1 and ich == 2)
                    for m in range(NM):
                        nc.tensor.matmul(
                            accs[m][:, h * 512:(h + 1) * 512],
                            w_dcn3_v[:, t * 3 + ich, m * P:(m + 1) * P],
                            smp[:, :, ich],
                            start=first, stop=last,
                        )
        # BN + SiLU in one activation per output tile
        for m in range(NM):
            nc.scalar.activation(z[m][:], accs[m][:], ACTF.Silu,
                                 bias=bn_shift[:, m:m + 1], scale=bn_scale[:, m:m + 1])

    # ---------- phase 8: 1x1 conv, pixel-major (output needs no transpose) ----------
    w2r_v = w2r[:].rearrange("c (k o) -> c k o", o=DIM)
    with tc.tile_pool(name="p8", bufs=3, space="PSUM") as p8_pool, \
         tc.tile_pool(name="osb", bufs=3) as osb_pool:
        for pt in range(NPT):
            ps = p8_pool.tile([P, DIM], F32)
            for kt in range(NCT):
                nc.tensor.matmul(
                    ps[:], z[kt][:, pt * P:(pt + 1) * P], w2r_v[:, kt, :],
                    start=(kt == 0), stop=False,
                )
            nc.tensor.matmul(ps[:], ones1[0:1, :], b2r[0:1, :],
                             start=False, stop=True)
            osb = osb_pool.tile([P, DIM], F32, tag="osb", name="osb")
            nc.scalar.activation(osb[:], ps[:], ACTF.Copy)
            nc.sync.dma_start(io["out"][pt * P:(pt + 1) * P, :], osb[:])


# ======================================================================
# SPMD entry point: full inputs in, full output out (8 cores, batch-parallel)
# ======================================================================

_PROGRAM_CACHE = {}


def _get_program(consts):
    key = "dcn2"
    if key not in _PROGRAM_CACHE:
        import concourse.bacc as bacc
        nc = bacc.Bacc("TRN2", target_bir_lowering=False, debug=False)
        io = declare_io(nc, consts)
        with tile.TileContext(nc) as tc:
            with ExitStack() as ctx:
                build(ctx, tc, io)
        nc.compile()
        _PROGRAM_CACHE[key] = nc
    return _PROGRAM_CACHE[key]


def kernel(**inputs) -> np.ndarray:
    from concourse.bass_utils import run_bass_kernel_spmd

    x = np.ascontiguousarray(np.asarray(inputs["x"], np.float32))
    B = x.shape[0]
    assert x.shape == (B, HW, DIM), x.shape
    consts = host_prep(inputs)
    nc = _get_program(consts)
    n_cores = 8
    reps = []
    for i in range(n_cores):
        m = {"x": x[i % B]}
        m.update(consts)
        reps.append(m)
    res = run_bass_kernel_spmd(nc, reps, list(range(n_cores)))
    out = np.stack([np.asarray(res.results[i]["out"], np.float32)
                    for i in range(B)], axis=0)
    return out
